# revision 1
# baseline (speedup 1.0000x reference)
# Trainium2 Bass kernel for nn_ExpertLinear (MoE grouped GEMM with routing).
#
# Strategy: data-parallel over tokens (8 cores), full weights replicated,
# fp16 compute with fp32 PSUM accumulation (measured rel err ~3.6e-4).
# Per core:
#   1. dma_gather(transpose=True) pulls the core's token rows from HBM x
#      (fp16) directly into the transposed [d_in, tokens] stationary-operand
#      layout, grouped by expert (per-expert groups padded to 128-row tiles,
#      tile counts shared across cores so one NEFF serves all 8).
#   2. Grouped GEMM per expert row-tile: 8 k-tile matmuls accumulate into
#      PSUM; eviction applies the per-row gate (DVE tensor_scalar) while
#      casting into an fp16 y buffer.
#   3. Combine: SBUF-source dma_gather (transpose mode) pulls each token's
#      two gated y rows (all gathers batched into dedicated tiles), DVE adds
#      produce out^T chunks written to DRAM. Host de-transposes and scatters
#      rows by the token->core assignment.
import os
import numpy as np

import concourse.bacc as bacc
import concourse.bass as bass
import concourse.mybir as mybir
import concourse.tile as tile
from concourse.bass_utils import run_bass_kernel_spmd

N_TOK = 8192
TOPK = 2
N_EXP = 8
D_IN = 1024
D_OUT = 1024
NCORES = 8
TPC = N_TOK // NCORES          # tokens per core
P = 128
KTILES = D_IN // P             # 8 k-tiles over d_in
F16 = mybir.dt.float16
F32 = mybir.dt.float32
I16 = mybir.dt.int16


def _pack16(flat):
    # [16, n/16] block (idx j at [j%16, j//16]), replicated into all eight
    # 16-partition groups — each GpSimd Q7 core reads its own copy.
    return np.ascontiguousarray(np.tile(flat.reshape(-1, 16).T, (8, 1)))


def _assign_cores(te):
    """Token->core assignment. Start from round-robin inside expert-pair
    groups, then rebalance so per-(core, expert) counts approach the
    ceil(global/8) optimum (minimizes the shared per-expert tile counts)."""
    pair_id = te[:, 0] * N_EXP + te[:, 1]
    order = np.argsort(pair_id, kind="stable")
    core_of_token = np.empty(N_TOK, np.int64)
    core_of_token[order] = np.arange(N_TOK) % NCORES

    gcnt = np.bincount(te.reshape(-1), minlength=N_EXP)
    target = -(-gcnt // NCORES)                      # ceil: per-core cap goal

    cnt = np.zeros((NCORES, N_EXP), np.int64)
    for c in range(NCORES):
        sel = core_of_token == c
        cnt[c] = np.bincount(te[sel].reshape(-1), minlength=N_EXP)

    # swap tokens between cores to cut per-expert maxima down to target
    for _ in range(4):
        done = True
        for e in range(N_EXP):
            over = np.where(cnt[:, e] > target[e])[0]
            for c in over:
                # tokens on core c touching e
                cand = np.where((core_of_token == c) & (te == e).any(axis=1))[0]
                for t in cand:
                    if cnt[c, e] <= target[e]:
                        break
                    e1, e2 = te[t]
                    # find a partner core with slack on both experts and a
                    # token to swap back that doesn't touch e
                    for c2 in range(NCORES):
                        if c2 == c or cnt[c2, e1] >= target[e1] or \
                           cnt[c2, e2] >= target[e2]:
                            continue
                        back = np.where((core_of_token == c2) &
                                        ~(te == e).any(axis=1))[0]
                        ok = None
                        for t2 in back:
                            f1, f2 = te[t2]
                            if cnt[c, f1] < target[f1] and \
                               cnt[c, f2] < target[f2]:
                                ok = t2
                                break
                        if ok is None:
                            continue
                        core_of_token[t] = c2
                        core_of_token[ok] = c
                        for x_ in (e1, e2):
                            cnt[c, x_] -= 1
                            cnt[c2, x_] += 1
                        f1, f2 = te[ok]
                        for x_ in (f1, f2):
                            cnt[c2, x_] -= 1
                            cnt[c, x_] += 1
                        done = False
                        break
        if done:
            break
    return core_of_token, cnt


def _plan(tok, sei, g_row):
    """Host routing plan. Returns per-core index/gate tables and the shared
    per-expert-slot tile counts T (max over cores, so one NEFF serves all)."""
    order_by_tok = np.argsort(tok, kind="stable")
    te = sei[order_by_tok].reshape(N_TOK, TOPK)

    core_of_token, cnt = _assign_cores(te)

    token_ids = [np.where(core_of_token == c)[0] for c in range(NCORES)]
    token_pos = np.empty(N_TOK, np.int64)
    for c in range(NCORES):
        token_pos[token_ids[c]] = np.arange(TPC)

    core_of_row = core_of_token[tok]
    rows_per_core = [np.where(core_of_row == c)[0] for c in range(NCORES)]

    # process experts smallest-first (fast first gather)
    perm = np.argsort(cnt.max(axis=0), kind="stable")
    T = np.maximum(1, -(-cnt.max(axis=0)[perm] // P))
    off = np.concatenate([[0], np.cumsum(T) * P])
    NP = int(off[-1])
    slot_of_expert = np.empty(N_EXP, np.int64)
    slot_of_expert[perm] = np.arange(N_EXP)

    per_core = []
    for c in range(NCORES):
        rows_c = rows_per_core[c]
        s_c = slot_of_expert[sei[rows_c]]
        ordr = np.argsort(s_c, kind="stable")
        rows_c = rows_c[ordr]
        s_c = s_c[ordr]
        scnt = np.bincount(s_c, minlength=N_EXP)
        within = np.arange(rows_c.size) - np.concatenate(
            [[0], np.cumsum(scnt)])[s_c]
        loc = off[s_c] + within

        gidx_flat = np.zeros(NP, np.int16)
        grow_flat = np.zeros(NP, np.float32)
        gidx_flat[loc] = tok[rows_c].astype(np.int16)
        grow_flat[loc] = g_row[rows_c]

        pos = token_pos[tok[rows_c]]
        r0_flat = np.zeros(TPC, np.int16)
        r1_flat = np.zeros(TPC, np.int16)
        seen = np.zeros(TPC, bool)
        for j in range(rows_c.size):
            p_ = pos[j]
            if seen[p_]:
                r1_flat[p_] = loc[j]
            else:
                r0_flat[p_] = loc[j]
                seen[p_] = True
        assert seen.all()

        per_core.append(
            dict(
                gidx=_pack16(gidx_flat),
                grow=np.ascontiguousarray(grow_flat.reshape(-1, P).T),
                r0i=_pack16(r0_flat),
                r1i=_pack16(r1_flat),
            )
        )
    return T, per_core, token_ids, perm


def _build_nc(T):
    NP = int(T.sum()) * P
    NB = NP // P
    off = np.concatenate([[0], np.cumsum(T)]) * P

    nc = bacc.Bacc("TRN2", target_bir_lowering=False, debug=False,
                   num_devices=NCORES)

    xh = nc.dram_tensor("xh", [N_TOK, D_IN], F16, kind="ExternalInput")
    wh = nc.dram_tensor("wh", [N_EXP, P, KTILES, D_OUT], F16,
                        kind="ExternalInput")
    gidx = nc.dram_tensor("gidx", [P, NP // 16], I16, kind="ExternalInput")
    grow = nc.dram_tensor("grow", [P, NB], F32, kind="ExternalInput")
    r0i = nc.dram_tensor("r0i", [P, TPC // 16], I16, kind="ExternalInput")
    r1i = nc.dram_tensor("r1i", [P, TPC // 16], I16, kind="ExternalInput")
    outT = nc.dram_tensor("outT", [P, D_OUT // P, TPC], F32,
                          kind="ExternalOutput")

    CH = 256
    NCH = TPC // CH

    # Pre-TileContext warmup: the first DMAGatherAnt triggers a ~15us Q7
    # extended-instruction library fetch; start it as early as possible so
    # it overlaps the preamble and the first input DMAs.
    warm_idx = nc.alloc_sbuf_tensor("warm_idx", [P, 8], I16)
    warm_dst = nc.alloc_sbuf_tensor("warm_dst", [P, P], F16)
    warm_sem = nc.alloc_semaphore("warm_set")
    warm_dma = nc.alloc_semaphore("warm_dma")
    nc.gpsimd.memset(warm_idx.ap(), 0).then_inc(warm_sem, 1)
    nc.gpsimd.wait_ge(warm_sem, 1)
    nc.gpsimd.dma_gather(
        warm_dst.ap().rearrange("p (a b) -> p a b", a=1),
        xh[:].rearrange("n (a b) -> (n a) b", b=P),
        warm_idx.ap(), num_idxs=P, num_idxs_reg=P, elem_size=P,
        transpose=True).then_inc(warm_dma, 16)

    with tile.TileContext(nc) as tc:
        with (
            tc.tile_pool(name="const", bufs=1) as kpool,
            tc.tile_pool(name="w", bufs=3) as wpool,
            tc.tile_pool(name="xT", bufs=1) as xpool,
            tc.tile_pool(name="y", bufs=1) as ypool,
            tc.tile_pool(name="cmb", bufs=1) as cpool,
            tc.tile_pool(name="ot", bufs=2) as opool,
            tc.tile_pool(name="ps", bufs=4, space="PSUM") as ppool,
        ):
            gidx_t = kpool.tile([P, NP // 16], I16)
            nc.sync.dma_start(gidx_t[:], gidx[:])

            # dispatch gathers up front (expert 0 split per row-tile so the
            # first matmul starts early)
            x_tiles = []
            for e in range(N_EXP):
                ne = int(T[e]) * P
                if e == 0:
                    parts = []
                    for t in range(int(T[e])):
                        xp = xpool.tile([P, KTILES, P], F16, tag=f"x0_{t}")
                        nc.gpsimd.dma_gather(
                            xp[:], xh[:],
                            gidx_t[:, (off[e] + t * P) // 16:
                                   (off[e] + (t + 1) * P) // 16],
                            num_idxs=P, num_idxs_reg=P, elem_size=D_IN,
                            transpose=True,
                        )
                        parts.append(xp)
                    x_tiles.append(parts)
                else:
                    x_t = xpool.tile([P, KTILES, ne], F16, tag=f"xT{e}")
                    nc.gpsimd.dma_gather(
                        x_t[:], xh[:],
                        gidx_t[:, off[e] // 16:(off[e] + ne) // 16],
                        num_idxs=ne, num_idxs_reg=ne, elem_size=D_IN,
                        transpose=True,
                    )
                    x_tiles.append(x_t)

            grow_t = kpool.tile([P, NB], F32)
            nc.sync.dma_start(grow_t[:], grow[:])
            r0_t = kpool.tile([P, TPC // 16], I16)
            nc.sync.dma_start(r0_t[:], r0i[:])
            r1_t = kpool.tile([P, TPC // 16], I16)
            nc.sync.dma_start(r1_t[:], r1i[:])

            y_t = ypool.tile([P, NB, D_OUT], F16)

            for e in range(N_EXP):
                w_t = wpool.tile([P, KTILES, D_OUT], F16, tag="w")
                for kk in range(KTILES):
                    nc.scalar.dma_start(w_t[:, kk], wh[e, :, kk])
                x_t = x_tiles[e]
                for t in range(int(T[e])):
                    rt_g = off[e] // P + t
                    ps0 = ppool.tile([P, 512], F32, tag="ps")
                    ps1 = ppool.tile([P, 512], F32, tag="ps")
                    for kk in range(KTILES):
                        if e == 0:
                            lhsT = x_t[t][:, kk, :]
                        else:
                            lhsT = x_t[:, kk, t * P:(t + 1) * P]
                        nc.tensor.matmul(ps0[:], lhsT, w_t[:, kk, 0:512],
                                         start=(kk == 0),
                                         stop=(kk == KTILES - 1))
                        nc.tensor.matmul(ps1[:], lhsT, w_t[:, kk, 512:1024],
                                         start=(kk == 0),
                                         stop=(kk == KTILES - 1))
                    gsc = grow_t[:, rt_g:rt_g + 1]
                    nc.vector.tensor_scalar_mul(y_t[:, rt_g, 0:512],
                                                ps0[:], gsc)
                    nc.vector.tensor_scalar_mul(y_t[:, rt_g, 512:1024],
                                                ps1[:], gsc)

            # combine: batch ALL gathers (dedicated tiles), then adds+stores.
            # Descending chunk sizes so the last transfer (which gates the
            # final add/store) is small.
            chunks = [384, 256, 256, 128]
            assert sum(chunks) == TPC
            gath, base = [], 0
            for h, ch in enumerate(chunks):
                g0 = cpool.tile([P, D_OUT // P, ch], F16, tag=f"c0_{h}",
                                name="g0")
                g1 = cpool.tile([P, D_OUT // P, ch], F16, tag=f"c1_{h}",
                                name="g1")
                for dst, ridx in ((g0, r0_t), (g1, r1_t)):
                    nc.gpsimd.dma_gather(
                        dst[:], y_t[:],
                        ridx[:, base // 16:(base + ch) // 16],
                        num_idxs=ch, num_idxs_reg=ch, elem_size=D_OUT,
                        transpose=True,
                        sbuf_tokens_per_rank=P,
                        sbuf_free_dim_per_rank=D_OUT * 2,
                    )
                gath.append((g0, g1, base, ch))
                base += ch
            for (g0, g1, base, ch) in gath:
                ot_full = opool.tile([P, D_OUT // P, 384], F32,
                                     tag="ot", name="ot")
                ot = ot_full[:, :, :ch]
                nc.vector.tensor_add(out=ot, in0=g0[:], in1=g1[:])
                nc.sync.dma_start(outT[:, :, base:base + ch], ot)

    nc.compile()
    return nc


def _prep(inputs):
    x = np.asarray(inputs["input"], np.float32)
    w = np.asarray(inputs["weight"], np.float32)
    k = int(np.asarray(inputs["k"]))
    assert k == TOPK
    sei = np.asarray(inputs["sorted_expert_indices"]).astype(np.int64)
    ssi = np.asarray(inputs["sorted_scattered_indices"]).astype(np.int64)
    gates = np.asarray(inputs["gates"], np.float32)

    tok = ssi // k
    g_row = gates.reshape(-1)[ssi]

    T, per_core, token_ids, perm = _plan(tok, sei, g_row)

    xh = x.astype(np.float16)
    whp = np.ascontiguousarray(
        w.reshape(N_EXP, KTILES, P, D_OUT).transpose(0, 2, 1, 3)
    ).astype(np.float16)[perm]

    in_maps = []
    for c in range(NCORES):
        m = dict(per_core[c])
        m["xh"] = xh
        m["wh"] = np.ascontiguousarray(whp)
        in_maps.append(m)
    return T, in_maps, token_ids


def _run(inputs, trace=False, trace_kwargs=None):
    T, in_maps, token_ids = _prep(inputs)
    nc = _build_nc(T)
    res = run_bass_kernel_spmd(
        nc, in_maps, core_ids=list(range(NCORES)), trace=trace,
        **(trace_kwargs or {}),
    )
    out = np.zeros((N_TOK, D_OUT), np.float32)
    for c in range(NCORES):
        oT = res.results[c]["outT"]
        out[token_ids[c]] = oT.transpose(2, 1, 0).reshape(TPC, D_OUT)
    return out, res


def kernel(**inputs) -> np.ndarray:
    out, _ = _run(inputs, trace=bool(int(os.environ.get("KERNEL_TRACE", "0"))))
    return out



# revision 6
# speedup vs baseline: 1.5177x; 1.5177x over previous
# Trainium2 Bass kernel for nn_ExpertLinear (MoE grouped GEMM with routing).
#
# Strategy: data-parallel over tokens (8 cores), full weights replicated,
# fp16 compute with fp32 PSUM accumulation.
# Per core:
#   1. dma_gather(transpose=True) pulls the core's token rows from HBM x
#      (fp16) into the transposed [d_in, rows] stationary-operand layout,
#      grouped by expert slot (per-slot groups padded to 128-row tiles, tile
#      counts shared across cores so one NEFF serves all 8). Tokens whose two
#      routed experts coincide are merged into ONE grouped row with gate
#      (g0+g1)/2 and combine indices r0==r1.
#   2. Weights stream continuously (one 2MB DMA per expert, deep prefetch).
#      Grouped GEMM per slot row-tile: 8 k-tile matmuls accumulate into two
#      512-wide PSUM halves; eviction applies the per-row gate (DVE
#      tensor_scalar) casting into an fp16 y buffer.
#   3. Combine happens in ROUNDS overlapped with the remaining matmuls: once
#      all slots <= b are evicted, the tokens whose experts all lie in slots
#      <= b are combined (SBUF-source transpose dma_gather of their two gated
#      y rows, DVE add, contiguous fp16 DRAM write). Host de-transposes and
#      scatters rows by the token->core assignment.
import os
import numpy as np

import concourse.bacc as bacc
import concourse.bass as bass
import concourse.mybir as mybir
import concourse.tile as tile
from concourse.bass_utils import run_bass_kernel_spmd

N_TOK = 8192
TOPK = 2
N_EXP = 8
D_IN = 1024
D_OUT = 1024
NCORES = 8
TPC = N_TOK // NCORES          # tokens per core
P = 128
KTILES = D_IN // P             # 8 k-tiles over d_in
F16 = mybir.dt.float16
F32 = mybir.dt.float32
I16 = mybir.dt.int16
ROUND_BNDS = (4, 5, 6, 7)      # combine-round slot boundaries


def _pack16(flat):
    # [16, n/16] block (idx j at [j%16, j//16]), replicated into all eight
    # 16-partition groups — each GpSimd Q7 core reads its own copy.
    return np.ascontiguousarray(np.tile(flat.reshape(-1, 16).T, (8, 1)))


def _assign_cores(e_lo, e_hi, merged):
    """Token->core assignment balancing per-(core, expert) row counts.
    Distributes each (e_lo, e_hi) pair type evenly across cores; leftovers
    placed greedily against per-expert targets while keeping exactly TPC
    tokens per core."""
    gcnt = np.bincount(e_lo, minlength=N_EXP) + np.bincount(
        e_hi[~merged], minlength=N_EXP)
    target = -(-gcnt // NCORES)

    core_of_token = np.full(N_TOK, -1, np.int64)
    cnt = np.zeros((NCORES, N_EXP), np.int64)
    ntok = np.zeros(NCORES, np.int64)
    leftovers = []
    type_key = e_lo * N_EXP + e_hi
    for ty in range(N_EXP * N_EXP):
        idxs = np.where(type_key == ty)[0]
        if idxs.size == 0:
            continue
        e1, e2 = divmod(ty, N_EXP)
        nfull = idxs.size // NCORES
        for c in range(NCORES):
            sel = idxs[c * nfull:(c + 1) * nfull]
            core_of_token[sel] = c
            ntok[c] += sel.size
            cnt[c, e1] += sel.size
            if e1 != e2:
                cnt[c, e2] += sel.size
        leftovers.extend(idxs[NCORES * nfull:].tolist())

    for t in leftovers:
        e1, e2 = e_lo[t], e_hi[t]
        best, bkey = -1, None
        for c in range(NCORES):
            if ntok[c] >= TPC:
                continue
            over = max(cnt[c, e1] + 1 - target[e1],
                       cnt[c, e2] + (1 if e1 != e2 else 0) - target[e2])
            key = (over, ntok[c], cnt[c, e1] + cnt[c, e2])
            if bkey is None or key < bkey:
                best, bkey = c, key
        core_of_token[t] = best
        ntok[best] += 1
        cnt[best, e1] += 1
        if e1 != e2:
            cnt[best, e2] += 1
    assert (ntok == TPC).all()
    return core_of_token, cnt


def _plan(tok, sei, g_row):
    """Host routing plan. Returns shared shapes (per-slot tile counts T,
    combine rounds) plus per-core index/gate tables and token ordering."""
    order_by_tok = np.argsort(tok, kind="stable")
    te = sei[order_by_tok].reshape(N_TOK, TOPK)
    tg = g_row[order_by_tok].reshape(N_TOK, TOPK)

    merged = te[:, 0] == te[:, 1]
    e_lo = te.min(axis=1)
    e_hi = te.max(axis=1)

    core_of_token, cnt = _assign_cores(e_lo, e_hi, merged)

    # slot order: ascending per-expert tile count (ties by expert id)
    T_exp = np.maximum(1, -(-cnt.max(axis=0) // P))
    perm = np.argsort(T_exp, kind="stable")          # slot -> expert
    T = T_exp[perm]
    slot_of_expert = np.empty(N_EXP, np.int64)
    slot_of_expert[perm] = np.arange(N_EXP)
    coff = np.concatenate([[0], np.cumsum(T)])       # chunks per slot bnd
    off = coff * P
    NP = int(off[-1])
    NB = NP // P

    # per-token slots (column i of te/tg pairs expert te[:,i] with gate tg[:,i])
    s_c0 = slot_of_expert[te[:, 0]]
    s_c1 = slot_of_expert[te[:, 1]]
    s_lo = np.minimum(s_c0, s_c1)
    s_hi = np.maximum(s_c0, s_c1)
    maxslot = s_hi

    # shared combine-round sizes: min over cores of ready-token counts
    ready = np.zeros((NCORES, len(ROUND_BNDS)), np.int64)
    for c in range(NCORES):
        ms_c = maxslot[core_of_token == c]
        for r, b in enumerate(ROUND_BNDS):
            ready[c, r] = (ms_c <= b).sum()
    cum = (ready.min(axis=0) // P) * P
    cum[-1] = TPC
    cum = np.maximum.accumulate(cum)
    rounds = []                                      # (size, base, bnd_chunk)
    base = 0
    for r, b in enumerate(ROUND_BNDS):
        size = int(cum[r]) - base
        if size <= 0:
            continue
        bnd = int(coff[b + 1])
        if r == len(ROUND_BNDS) - 1 and size >= 256:
            rounds.append((size - P, base, bnd))
            rounds.append((P, base + size - P, bnd))
        else:
            rounds.append((size, base, bnd))
        base += size
    assert base == TPC

    per_core = []
    token_ids = []
    for c in range(NCORES):
        toks_c = np.where(core_of_token == c)[0]
        # grouped rows: (slot, token, gate); merged tokens get one row with
        # half-sum gate and r0 == r1 (so the combine add reconstitutes g0+g1)
        n1 = int(merged[toks_c].sum())
        nrows = 2 * toks_c.size - n1
        r_slot = np.empty(nrows, np.int64)
        r_tok = np.empty(nrows, np.int64)
        r_gate = np.empty(nrows, np.float64)
        r_which = np.empty(nrows, np.int64)          # 0: lo slot, 1: hi slot
        i = 0
        for t in toks_c:
            if merged[t]:
                r_slot[i] = s_lo[t]
                r_tok[i] = t
                r_gate[i] = (tg[t, 0] + tg[t, 1]) * 0.5
                r_which[i] = 0
                i += 1
            else:
                if s_c0[t] <= s_c1[t]:
                    lo_gate, hi_gate = tg[t, 0], tg[t, 1]
                else:
                    lo_gate, hi_gate = tg[t, 1], tg[t, 0]
                r_slot[i] = s_lo[t]; r_tok[i] = t
                r_gate[i] = lo_gate; r_which[i] = 0
                i += 1
                r_slot[i] = s_hi[t]; r_tok[i] = t
                r_gate[i] = hi_gate; r_which[i] = 1
                i += 1
        assert i == nrows

        ordr = np.argsort(r_slot, kind="stable")
        r_slot = r_slot[ordr]; r_tok = r_tok[ordr]
        r_gate = r_gate[ordr]; r_which = r_which[ordr]
        scnt = np.bincount(r_slot, minlength=N_EXP)
        assert (scnt <= T * P).all()
        within = np.arange(nrows) - np.concatenate([[0], np.cumsum(scnt)])[r_slot]
        loc = off[r_slot] + within

        gidx_flat = np.zeros(NP, np.int16)
        grow_flat = np.zeros(NP, np.float32)
        gidx_flat[loc] = r_tok.astype(np.int16)
        grow_flat[loc] = r_gate

        # combine row indices per token
        r0_of = np.full(N_TOK, -1, np.int64)
        r1_of = np.full(N_TOK, -1, np.int64)
        lo_mask = r_which == 0
        r0_of[r_tok[lo_mask]] = loc[lo_mask]
        r1_of[r_tok[~lo_mask]] = loc[~lo_mask]
        mm = merged[toks_c]
        r1_of[toks_c[mm]] = r0_of[toks_c[mm]]
        assert (r0_of[toks_c] >= 0).all() and (r1_of[toks_c] >= 0).all()

        # token ordering by combine round
        ms_c = maxslot[toks_c]
        order = np.argsort(ms_c * N_TOK + toks_c, kind="stable")
        toks_sorted = toks_c[order]
        # validate against round feasibility
        mss = ms_c[order]
        pos = 0
        for (size, bse, bnd) in rounds:
            need_slot = [b for b in ROUND_BNDS if coff[b + 1] == bnd][0]
            assert (mss[pos:pos + size] <= need_slot).all()
            pos += size
        token_ids.append(toks_sorted)

        r0_flat = r0_of[toks_sorted].astype(np.int16)
        r1_flat = r1_of[toks_sorted].astype(np.int16)

        per_core.append(
            dict(
                gidx=_pack16(gidx_flat),
                grow=np.ascontiguousarray(grow_flat.reshape(-1, P).T),
                r0i=_pack16(r0_flat),
                r1i=_pack16(r1_flat),
            )
        )
    return T, rounds, per_core, token_ids, perm


def _build_nc(T, rounds):
    NB = int(T.sum())
    NP = NB * P
    coff = np.concatenate([[0], np.cumsum(T)])
    off = coff * P

    nc = bacc.Bacc("TRN2", target_bir_lowering=False, debug=False,
                   num_devices=NCORES)

    xh = nc.dram_tensor("xh", [N_TOK, D_IN], F16, kind="ExternalInput")
    wh = nc.dram_tensor("wh", [N_EXP, P, KTILES, D_OUT], F16,
                        kind="ExternalInput")
    gidx = nc.dram_tensor("gidx", [P, NP // 16], I16, kind="ExternalInput")
    grow = nc.dram_tensor("grow", [P, NB], F32, kind="ExternalInput")
    r0i = nc.dram_tensor("r0i", [P, TPC // 16], I16, kind="ExternalInput")
    r1i = nc.dram_tensor("r1i", [P, TPC // 16], I16, kind="ExternalInput")
    outR = [
        nc.dram_tensor(f"outR{r}", [P, D_OUT // P, size], F16,
                       kind="ExternalOutput")
        for r, (size, _, _) in enumerate(rounds)
    ]

    # Pre-TileContext warmup: the first DMAGatherAnt triggers a ~15us Q7
    # extended-instruction library fetch; start it as early as possible so
    # it overlaps the preamble and the first input DMAs.
    warm_idx = nc.alloc_sbuf_tensor("warm_idx", [P, 8], I16)
    warm_dst = nc.alloc_sbuf_tensor("warm_dst", [P, P], F16)
    warm_sem = nc.alloc_semaphore("warm_set")
    warm_dma = nc.alloc_semaphore("warm_dma")
    nc.gpsimd.memset(warm_idx.ap(), 0).then_inc(warm_sem, 1)
    nc.gpsimd.wait_ge(warm_sem, 1)
    nc.gpsimd.dma_gather(
        warm_dst.ap().rearrange("p (a b) -> p a b", a=1),
        xh[:].rearrange("n (a b) -> (n a) b", b=P),
        warm_idx.ap(), num_idxs=P, num_idxs_reg=P, elem_size=P,
        transpose=True).then_inc(warm_dma, 16)

    rounds_after_slot = {}
    for r, (size, base, bnd) in enumerate(rounds):
        s = int(np.searchsorted(coff, bnd, side="left")) - 1
        rounds_after_slot.setdefault(s, []).append((r, size, base, bnd))

    with tile.TileContext(nc) as tc:
        with (
            tc.tile_pool(name="const", bufs=1) as kpool,
            tc.tile_pool(name="w", bufs=4) as wpool,
            tc.tile_pool(name="xT", bufs=1) as xpool,
            tc.tile_pool(name="y", bufs=1) as ypool,
            tc.tile_pool(name="cmb", bufs=2) as cpool,
            tc.tile_pool(name="ot", bufs=2) as opool,
            tc.tile_pool(name="ps", bufs=8, space="PSUM") as ppool,
        ):
            gidx_t = kpool.tile([P, NP // 16], I16)
            nc.sync.dma_start(gidx_t[:], gidx[:])

            # dispatch gathers up front (slot 0 split per row-tile so the
            # first matmul starts early)
            x_tiles = []
            for s in range(N_EXP):
                ne = int(T[s]) * P
                if s == 0:
                    parts = []
                    for t in range(int(T[s])):
                        xp = xpool.tile([P, KTILES, P], F16, tag=f"x0_{t}")
                        nc.gpsimd.dma_gather(
                            xp[:], xh[:],
                            gidx_t[:, (off[s] + t * P) // 16:
                                   (off[s] + (t + 1) * P) // 16],
                            num_idxs=P, num_idxs_reg=P, elem_size=D_IN,
                            transpose=True,
                        )
                        parts.append(xp)
                    x_tiles.append(parts)
                else:
                    x_t = xpool.tile([P, KTILES, ne], F16, tag=f"xT{s}")
                    nc.gpsimd.dma_gather(
                        x_t[:], xh[:],
                        gidx_t[:, off[s] // 16:(off[s] + ne) // 16],
                        num_idxs=ne, num_idxs_reg=ne, elem_size=D_IN,
                        transpose=True,
                    )
                    x_tiles.append(x_t)

            grow_t = kpool.tile([P, NB], F32)
            nc.sync.dma_start(grow_t[:], grow[:])
            r0_t = kpool.tile([P, TPC // 16], I16)
            nc.sync.dma_start(r0_t[:], r0i[:])
            r1_t = kpool.tile([P, TPC // 16], I16)
            nc.sync.dma_start(r1_t[:], r1i[:])

            y_t = ypool.tile([P, NB, D_OUT], F16)

            split_w = bool(int(os.environ.get("KV_SPLITW", "0")))
            for s in range(N_EXP):
                w_t = wpool.tile([P, KTILES, D_OUT], F16, tag="w")
                if split_w:
                    for kk in range(KTILES):
                        nc.scalar.dma_start(w_t[:, kk], wh[s, :, kk])
                else:
                    nc.scalar.dma_start(w_t[:], wh[s])
                x_t = x_tiles[s]
                for t in range(int(T[s])):
                    rt_g = int(coff[s]) + t
                    ps0 = ppool.tile([P, 512], F32, tag="ps")
                    ps1 = ppool.tile([P, 512], F32, tag="ps")
                    for kk in range(KTILES):
                        if s == 0:
                            lhsT = x_t[t][:, kk, :]
                        else:
                            lhsT = x_t[:, kk, t * P:(t + 1) * P]
                        nc.tensor.matmul(ps0[:], lhsT, w_t[:, kk, 0:512],
                                         start=(kk == 0),
                                         stop=(kk == KTILES - 1))
                        nc.tensor.matmul(ps1[:], lhsT, w_t[:, kk, 512:1024],
                                         start=(kk == 0),
                                         stop=(kk == KTILES - 1))
                    gsc = grow_t[:, rt_g:rt_g + 1]
                    nc.vector.tensor_scalar_mul(y_t[:, rt_g, 0:512],
                                                ps0[:], gsc)
                    nc.vector.tensor_scalar_mul(y_t[:, rt_g, 512:1024],
                                                ps1[:], gsc)

                # combine rounds whose tokens are fully evicted by now
                for (r, size, base, bnd) in rounds_after_slot.get(s, []):
                    g0 = cpool.tile([P, D_OUT // P, size], F16,
                                    tag=f"g0_{size}", name="g0")
                    g1 = cpool.tile([P, D_OUT // P, size], F16,
                                    tag=f"g1_{size}", name="g1")
                    src = (y_t[:] if int(os.environ.get("KV_FULLVIEW", "0"))
                           else y_t[:, :bnd, :])
                    for dst, ridx in ((g0, r0_t), (g1, r1_t)):
                        nc.gpsimd.dma_gather(
                            dst[:], src,
                            ridx[:, base // 16:(base + size) // 16],
                            num_idxs=size, num_idxs_reg=size,
                            elem_size=D_OUT, transpose=True,
                            sbuf_tokens_per_rank=P,
                            sbuf_free_dim_per_rank=D_OUT * 2,
                        )
                    ot = opool.tile([P, D_OUT // P, size], F16,
                                    tag=f"ot_{size}", name="ot")
                    nc.vector.tensor_add(out=ot[:], in0=g0[:], in1=g1[:])
                    nc.sync.dma_start(outR[r][:], ot[:])

    nc.compile()
    return nc


def _prep(inputs):
    x = np.asarray(inputs["input"], np.float32)
    w = np.asarray(inputs["weight"], np.float32)
    k = int(np.asarray(inputs["k"]))
    assert k == TOPK
    sei = np.asarray(inputs["sorted_expert_indices"]).astype(np.int64)
    ssi = np.asarray(inputs["sorted_scattered_indices"]).astype(np.int64)
    gates = np.asarray(inputs["gates"], np.float32)

    tok = ssi // k
    g_row = gates.reshape(-1)[ssi]

    T, rounds, per_core, token_ids, perm = _plan(tok, sei, g_row)

    xh = x.astype(np.float16)
    whp = np.ascontiguousarray(
        w.reshape(N_EXP, KTILES, P, D_OUT).transpose(0, 2, 1, 3)
    ).astype(np.float16)[perm]

    in_maps = []
    for c in range(NCORES):
        m = dict(per_core[c])
        m["xh"] = xh
        m["wh"] = np.ascontiguousarray(whp)
        in_maps.append(m)
    return T, rounds, in_maps, token_ids


def _run(inputs, trace=False, trace_kwargs=None):
    T, rounds, in_maps, token_ids = _prep(inputs)
    nc = _build_nc(T, rounds)
    res = run_bass_kernel_spmd(
        nc, in_maps, core_ids=list(range(NCORES)), trace=trace,
        **(trace_kwargs or {}),
    )
    out = np.zeros((N_TOK, D_OUT), np.float32)
    for c in range(NCORES):
        for r, (size, base, bnd) in enumerate(rounds):
            oT = res.results[c][f"outR{r}"]          # [P, 8, size] f16
            rows = oT.transpose(2, 1, 0).reshape(size, D_OUT)
            out[token_ids[c][base:base + size]] = rows.astype(np.float32)
    return out, res


def kernel(**inputs) -> np.ndarray:
    out, _ = _run(inputs, trace=bool(int(os.environ.get("KERNEL_TRACE", "0"))))
    return out


# revision 12
# speedup vs baseline: 1.5478x; 1.0198x over previous
# Trainium2 Bass kernel for nn_ExpertLinear (MoE grouped GEMM with routing).
#
# Strategy: data-parallel over tokens (8 cores), full weights replicated,
# fp16 compute with fp32 PSUM accumulation.
# Per core:
#   1. dma_gather(transpose=True) pulls the core's token rows from HBM x
#      (fp16) into the transposed [d_in, rows] stationary-operand layout,
#      grouped by expert slot (per-slot groups padded to 128-row tiles, tile
#      counts shared across cores so one NEFF serves all 8). Tokens whose two
#      routed experts coincide are merged into ONE grouped row with gate
#      (g0+g1)/2 and combine indices r0==r1.
#   2. Weights stream continuously (one 2MB DMA per expert, deep prefetch).
#      Grouped GEMM per slot row-tile: 8 k-tile matmuls accumulate into two
#      512-wide PSUM halves; eviction applies the per-row gate (DVE
#      tensor_scalar) casting into an fp16 y buffer.
#   3. Combine happens in ROUNDS overlapped with the remaining matmuls: once
#      all slots <= b are evicted, the tokens whose experts all lie in slots
#      <= b are combined (SBUF-source transpose dma_gather of their two gated
#      y rows, DVE add, contiguous fp16 DRAM write). Host de-transposes and
#      scatters rows by the token->core assignment.
import os
import numpy as np

import concourse.bacc as bacc
import concourse.bass as bass
import concourse.mybir as mybir
import concourse.tile as tile
from concourse.bass_utils import run_bass_kernel_spmd

N_TOK = 8192
TOPK = 2
N_EXP = 8
D_IN = 1024
D_OUT = 1024
NCORES = 8
TPC = N_TOK // NCORES          # tokens per core
P = 128
KTILES = D_IN // P             # 8 k-tiles over d_in
F16 = mybir.dt.float16
F32 = mybir.dt.float32
I16 = mybir.dt.int16
ROUND_BNDS = (4, 5, 6, 7)      # combine-round slot boundaries


def _pack16(flat):
    # [16, n/16] block (idx j at [j%16, j//16]), replicated into all eight
    # 16-partition groups — each GpSimd Q7 core reads its own copy.
    return np.ascontiguousarray(np.tile(flat.reshape(-1, 16).T, (8, 1)))


def _assign_cores(e_lo, e_hi, merged):
    """Token->core assignment balancing per-(core, expert) row counts.
    Distributes each (e_lo, e_hi) pair type evenly across cores; leftovers
    placed greedily against per-expert targets while keeping exactly TPC
    tokens per core."""
    gcnt = np.bincount(e_lo, minlength=N_EXP) + np.bincount(
        e_hi[~merged], minlength=N_EXP)
    target = -(-gcnt // NCORES)

    core_of_token = np.full(N_TOK, -1, np.int64)
    cnt = np.zeros((NCORES, N_EXP), np.int64)
    ntok = np.zeros(NCORES, np.int64)
    leftovers = []
    type_key = e_lo * N_EXP + e_hi
    for ty in range(N_EXP * N_EXP):
        idxs = np.where(type_key == ty)[0]
        if idxs.size == 0:
            continue
        e1, e2 = divmod(ty, N_EXP)
        nfull = idxs.size // NCORES
        for c in range(NCORES):
            sel = idxs[c * nfull:(c + 1) * nfull]
            core_of_token[sel] = c
            ntok[c] += sel.size
            cnt[c, e1] += sel.size
            if e1 != e2:
                cnt[c, e2] += sel.size
        leftovers.extend(idxs[NCORES * nfull:].tolist())

    for t in leftovers:
        e1, e2 = e_lo[t], e_hi[t]
        best, bkey = -1, None
        for c in range(NCORES):
            if ntok[c] >= TPC:
                continue
            over = max(cnt[c, e1] + 1 - target[e1],
                       cnt[c, e2] + (1 if e1 != e2 else 0) - target[e2])
            key = (over, ntok[c], cnt[c, e1] + cnt[c, e2])
            if bkey is None or key < bkey:
                best, bkey = c, key
        core_of_token[t] = best
        ntok[best] += 1
        cnt[best, e1] += 1
        if e1 != e2:
            cnt[best, e2] += 1
    assert (ntok == TPC).all()
    return core_of_token, cnt


def _plan(tok, sei, g_row):
    """Host routing plan. Returns shared shapes (per-slot tile counts T,
    combine rounds) plus per-core index/gate tables and token ordering."""
    order_by_tok = np.argsort(tok, kind="stable")
    te = sei[order_by_tok].reshape(N_TOK, TOPK)
    tg = g_row[order_by_tok].reshape(N_TOK, TOPK)

    merged = te[:, 0] == te[:, 1]
    e_lo = te.min(axis=1)
    e_hi = te.max(axis=1)

    core_of_token, cnt = _assign_cores(e_lo, e_hi, merged)

    # slot order: ascending per-expert tile count (ties by expert id)
    T_exp = np.maximum(1, -(-cnt.max(axis=0) // P))
    perm = np.argsort(T_exp, kind="stable")          # slot -> expert
    T = T_exp[perm]
    slot_of_expert = np.empty(N_EXP, np.int64)
    slot_of_expert[perm] = np.arange(N_EXP)
    coff = np.concatenate([[0], np.cumsum(T)])       # chunks per slot bnd
    off = coff * P
    NP = int(off[-1])
    NB = NP // P

    # per-token slots (column i of te/tg pairs expert te[:,i] with gate tg[:,i])
    s_c0 = slot_of_expert[te[:, 0]]
    s_c1 = slot_of_expert[te[:, 1]]
    s_lo = np.minimum(s_c0, s_c1)
    s_hi = np.maximum(s_c0, s_c1)
    maxslot = s_hi

    # shared combine-round sizes: min over cores of ready-token counts
    ready = np.zeros((NCORES, len(ROUND_BNDS)), np.int64)
    for c in range(NCORES):
        ms_c = maxslot[core_of_token == c]
        for r, b in enumerate(ROUND_BNDS):
            ready[c, r] = (ms_c <= b).sum()
    cum = (ready.min(axis=0) // P) * P
    cum[-1] = TPC
    cum = np.maximum.accumulate(cum)
    rounds = []                                      # (size, base, bnd_chunk)
    base = 0
    for r, b in enumerate(ROUND_BNDS):
        size = int(cum[r]) - base
        if size <= 0:
            continue
        bnd = int(coff[b + 1])
        rounds.append((size, base, bnd))
        base += size
    assert base == TPC

    per_core = []
    token_ids = []
    for c in range(NCORES):
        toks_c = np.where(core_of_token == c)[0]
        # grouped rows: (slot, token, gate); merged tokens get one row with
        # half-sum gate and r0 == r1 (so the combine add reconstitutes g0+g1)
        n1 = int(merged[toks_c].sum())
        nrows = 2 * toks_c.size - n1
        r_slot = np.empty(nrows, np.int64)
        r_tok = np.empty(nrows, np.int64)
        r_gate = np.empty(nrows, np.float64)
        r_which = np.empty(nrows, np.int64)          # 0: lo slot, 1: hi slot
        i = 0
        for t in toks_c:
            if merged[t]:
                r_slot[i] = s_lo[t]
                r_tok[i] = t
                r_gate[i] = (tg[t, 0] + tg[t, 1]) * 0.5
                r_which[i] = 0
                i += 1
            else:
                if s_c0[t] <= s_c1[t]:
                    lo_gate, hi_gate = tg[t, 0], tg[t, 1]
                else:
                    lo_gate, hi_gate = tg[t, 1], tg[t, 0]
                r_slot[i] = s_lo[t]; r_tok[i] = t
                r_gate[i] = lo_gate; r_which[i] = 0
                i += 1
                r_slot[i] = s_hi[t]; r_tok[i] = t
                r_gate[i] = hi_gate; r_which[i] = 1
                i += 1
        assert i == nrows

        ordr = np.argsort(r_slot, kind="stable")
        r_slot = r_slot[ordr]; r_tok = r_tok[ordr]
        r_gate = r_gate[ordr]; r_which = r_which[ordr]
        scnt = np.bincount(r_slot, minlength=N_EXP)
        assert (scnt <= T * P).all()
        within = np.arange(nrows) - np.concatenate([[0], np.cumsum(scnt)])[r_slot]
        loc = off[r_slot] + within

        gidx_flat = np.zeros(NP, np.int16)
        grow_flat = np.zeros(NP, np.float32)
        gidx_flat[loc] = r_tok.astype(np.int16)
        grow_flat[loc] = r_gate

        # combine row indices per token
        r0_of = np.full(N_TOK, -1, np.int64)
        r1_of = np.full(N_TOK, -1, np.int64)
        lo_mask = r_which == 0
        r0_of[r_tok[lo_mask]] = loc[lo_mask]
        r1_of[r_tok[~lo_mask]] = loc[~lo_mask]
        mm = merged[toks_c]
        r1_of[toks_c[mm]] = r0_of[toks_c[mm]]
        assert (r0_of[toks_c] >= 0).all() and (r1_of[toks_c] >= 0).all()

        # token ordering by combine round
        ms_c = maxslot[toks_c]
        order = np.argsort(ms_c * N_TOK + toks_c, kind="stable")
        toks_sorted = toks_c[order]
        # validate against round feasibility
        mss = ms_c[order]
        pos = 0
        for (size, bse, bnd) in rounds:
            need_slot = [b for b in ROUND_BNDS if coff[b + 1] == bnd][0]
            assert (mss[pos:pos + size] <= need_slot).all()
            pos += size
        token_ids.append(toks_sorted)

        r0_flat = r0_of[toks_sorted].astype(np.int16)
        r1_flat = r1_of[toks_sorted].astype(np.int16)

        per_core.append(
            dict(
                gidx=_pack16(gidx_flat),
                grow=np.ascontiguousarray(grow_flat.reshape(-1, P).T),
                r0i=_pack16(r0_flat),
                r1i=_pack16(r1_flat),
            )
        )
    return T, rounds, per_core, token_ids, perm


def _build_nc(T, rounds):
    NB = int(T.sum())
    NP = NB * P
    coff = np.concatenate([[0], np.cumsum(T)])
    off = coff * P

    nc = bacc.Bacc("TRN2", target_bir_lowering=False, debug=False,
                   num_devices=NCORES)

    xh = nc.dram_tensor("xh", [N_TOK, D_IN], F16, kind="ExternalInput")
    wh = nc.dram_tensor("wh", [N_EXP, P, KTILES, D_OUT], F16,
                        kind="ExternalInput")
    gidx = nc.dram_tensor("gidx", [P, NP // 16], I16, kind="ExternalInput")
    grow = nc.dram_tensor("grow", [P, NB], F32, kind="ExternalInput")
    r0i = nc.dram_tensor("r0i", [P, TPC // 16], I16, kind="ExternalInput")
    r1i = nc.dram_tensor("r1i", [P, TPC // 16], I16, kind="ExternalInput")
    outR = [
        nc.dram_tensor(f"outR{r}", [P, D_OUT // P, size], F16,
                       kind="ExternalOutput")
        for r, (size, _, _) in enumerate(rounds)
    ]

    # Pre-TileContext warmup: the first DMAGatherAnt triggers a ~15us Q7
    # extended-instruction library fetch; start it as early as possible so
    # it overlaps the preamble and the first input DMAs.
    warm_idx = nc.alloc_sbuf_tensor("warm_idx", [P, 8], I16)
    warm_dst = nc.alloc_sbuf_tensor("warm_dst", [P, P], F16)
    warm_sem = nc.alloc_semaphore("warm_set")
    warm_dma = nc.alloc_semaphore("warm_dma")
    nc.gpsimd.memset(warm_idx.ap(), 0).then_inc(warm_sem, 1)
    nc.gpsimd.wait_ge(warm_sem, 1)
    nc.gpsimd.dma_gather(
        warm_dst.ap().rearrange("p (a b) -> p a b", a=1),
        xh[:].rearrange("n (a b) -> (n a) b", b=P),
        warm_idx.ap(), num_idxs=P, num_idxs_reg=P, elem_size=P,
        transpose=True).then_inc(warm_dma, 16)

    rounds_after_slot = {}
    for r, (size, base, bnd) in enumerate(rounds):
        s = int(np.searchsorted(coff, bnd, side="left")) - 1
        rounds_after_slot.setdefault(s, []).append((r, size, base, bnd))

    with tile.TileContext(nc) as tc:
        with (
            tc.tile_pool(name="const", bufs=1) as kpool,
            tc.tile_pool(name="w", bufs=4) as wpool,
            tc.tile_pool(name="xT", bufs=1) as xpool,
            tc.tile_pool(name="y", bufs=1) as ypool,
            tc.tile_pool(name="cmb", bufs=2) as cpool,
            tc.tile_pool(name="ot", bufs=2) as opool,
            tc.tile_pool(name="ps", bufs=8, space="PSUM") as ppool,
        ):
            gidx_t = kpool.tile([P, NP // 16], I16)
            nc.sync.dma_start(gidx_t[:], gidx[:])

            # dispatch gathers up front (slot 0 split per row-tile so the
            # first matmul starts early)
            x_tiles = []
            for s in range(N_EXP):
                ne = int(T[s]) * P
                if s == 0:
                    parts = []
                    for t in range(int(T[s])):
                        xp = xpool.tile([P, KTILES, P], F16, tag=f"x0_{t}")
                        nc.gpsimd.dma_gather(
                            xp[:], xh[:],
                            gidx_t[:, (off[s] + t * P) // 16:
                                   (off[s] + (t + 1) * P) // 16],
                            num_idxs=P, num_idxs_reg=P, elem_size=D_IN,
                            transpose=True,
                        )
                        parts.append(xp)
                    x_tiles.append(parts)
                else:
                    x_t = xpool.tile([P, KTILES, ne], F16, tag=f"xT{s}")
                    nc.gpsimd.dma_gather(
                        x_t[:], xh[:],
                        gidx_t[:, off[s] // 16:(off[s] + ne) // 16],
                        num_idxs=ne, num_idxs_reg=ne, elem_size=D_IN,
                        transpose=True,
                    )
                    x_tiles.append(x_t)

            grow_t = kpool.tile([P, NB], F32)
            nc.sync.dma_start(grow_t[:], grow[:])
            r0_t = kpool.tile([P, TPC // 16], I16)
            nc.sync.dma_start(r0_t[:], r0i[:])
            r1_t = kpool.tile([P, TPC // 16], I16)
            nc.sync.dma_start(r1_t[:], r1i[:])

            y_t = ypool.tile([P, NB, D_OUT], F16)

            for s in range(N_EXP):
                w_t = wpool.tile([P, KTILES, D_OUT], F16, tag="w")
                if s == 2:
                    # Keep the Q7 extended-library fetch and the first x
                    # gathers off the weight stream's back: only w0/w1
                    # compete with them.  The dummy ACT copy reads the first
                    # x tile, so on the FIFO scalar queue w2..w7 only start
                    # once the library is loaded and x0 has landed.
                    gate_t = kpool.tile([P, 1], F16)
                    nc.scalar.activation(gate_t[:], x_tiles[0][0][:, 0, 0:1],
                                         mybir.ActivationFunctionType.Copy)
                nc.scalar.dma_start(w_t[:], wh[s])
                x_t = x_tiles[s]
                for t in range(int(T[s])):
                    rt_g = int(coff[s]) + t
                    ps0 = ppool.tile([P, 512], F32, tag="ps")
                    ps1 = ppool.tile([P, 512], F32, tag="ps")
                    for kk in range(KTILES):
                        if s == 0:
                            lhsT = x_t[t][:, kk, :]
                        else:
                            lhsT = x_t[:, kk, t * P:(t + 1) * P]
                        nc.tensor.matmul(ps0[:], lhsT, w_t[:, kk, 0:512],
                                         start=(kk == 0),
                                         stop=(kk == KTILES - 1))
                        nc.tensor.matmul(ps1[:], lhsT, w_t[:, kk, 512:1024],
                                         start=(kk == 0),
                                         stop=(kk == KTILES - 1))
                    gsc = grow_t[:, rt_g:rt_g + 1]
                    nc.vector.tensor_scalar_mul(y_t[:, rt_g, 0:512],
                                                ps0[:], gsc)
                    nc.vector.tensor_scalar_mul(y_t[:, rt_g, 512:1024],
                                                ps1[:], gsc)

                # combine rounds whose tokens are fully evicted by now; the
                # partial y view keeps each round's gathers off the later
                # slots' eviction path.
                for (r, size, base, bnd) in rounds_after_slot.get(s, []):
                    g0 = cpool.tile([P, D_OUT // P, size], F16,
                                    tag=f"g0_{size}", name="g0")
                    g1 = cpool.tile([P, D_OUT // P, size], F16,
                                    tag=f"g1_{size}", name="g1")
                    for dst, ridx in ((g0, r0_t), (g1, r1_t)):
                        nc.gpsimd.dma_gather(
                            dst[:], y_t[:, :bnd, :],
                            ridx[:, base // 16:(base + size) // 16],
                            num_idxs=size, num_idxs_reg=size,
                            elem_size=D_OUT, transpose=True,
                            sbuf_tokens_per_rank=P,
                            sbuf_free_dim_per_rank=D_OUT * 2,
                        )
                    ot = opool.tile([P, D_OUT // P, size], F16,
                                    tag=f"ot_{size}", name="ot")
                    nc.vector.tensor_add(out=ot[:], in0=g0[:], in1=g1[:])
                    nc.sync.dma_start(outR[r][:], ot[:])

    nc.compile()
    return nc


def _prep(inputs):
    x = np.asarray(inputs["input"], np.float32)
    w = np.asarray(inputs["weight"], np.float32)
    k = int(np.asarray(inputs["k"]))
    assert k == TOPK
    sei = np.asarray(inputs["sorted_expert_indices"]).astype(np.int64)
    ssi = np.asarray(inputs["sorted_scattered_indices"]).astype(np.int64)
    gates = np.asarray(inputs["gates"], np.float32)

    tok = ssi // k
    g_row = gates.reshape(-1)[ssi]

    T, rounds, per_core, token_ids, perm = _plan(tok, sei, g_row)

    xh = x.astype(np.float16)
    whp = np.ascontiguousarray(
        w.reshape(N_EXP, KTILES, P, D_OUT).transpose(0, 2, 1, 3)
    ).astype(np.float16)[perm]

    in_maps = []
    for c in range(NCORES):
        m = dict(per_core[c])
        m["xh"] = xh
        m["wh"] = np.ascontiguousarray(whp)
        in_maps.append(m)
    return T, rounds, in_maps, token_ids


def _run(inputs, trace=False, trace_kwargs=None):
    T, rounds, in_maps, token_ids = _prep(inputs)
    nc = _build_nc(T, rounds)
    res = run_bass_kernel_spmd(
        nc, in_maps, core_ids=list(range(NCORES)), trace=trace,
        **(trace_kwargs or {}),
    )
    out = np.zeros((N_TOK, D_OUT), np.float32)
    for c in range(NCORES):
        for r, (size, base, bnd) in enumerate(rounds):
            oT = res.results[c][f"outR{r}"]          # [P, 8, size] f16
            rows = oT.transpose(2, 1, 0).reshape(size, D_OUT)
            out[token_ids[c][base:base + size]] = rows.astype(np.float32)
    return out, res


def kernel(**inputs) -> np.ndarray:
    out, _ = _run(inputs, trace=bool(int(os.environ.get("KERNEL_TRACE", "0"))))
    return out
